# revision 10
# baseline (speedup 1.0000x reference)
"""SSIM loss Bass/Tile kernel for Trainium2, data-parallel over 8 NeuronCores.

Math: the reference's 23x23 Gaussian depthwise conv is exactly separable
(weight = outer(g, g) with g = weight.sum(axis=1)), so each 2D blur is two
1D 23-tap convolutions, each expressed as banded-Toeplitz matmuls on the
tensor engine ("data as lhsT": out = data_chunk.T @ band, which convolves
the partition dim and transposes the layout; two passes restore layout).

Blurred maps per plane (4): ms=blur(x+y), md=blur(x-y), d2b=blur(2xy),
s2b=blur(x^2+y^2). Then with A=ms^2, B=md^2 (scaled to undo bf16 tap-sum
error): N1=A-B+2C1, D1=A+B+2C1, N2=2*d2b+2C2-(A-B), D2=2*s2b+2C2-(A+B)
give ssim = N1*N2/(D1*D2) (all terms are 2x the reference's, which cancels).
Per-core partial sums are reduced on host: loss = 1 - sum/count.
"""

import numpy as np
import ml_dtypes

import concourse.bass as bass
import concourse.tile as tile
from concourse import bacc, mybir
from concourse.bass_utils import run_bass_kernel_spmd

BF16 = ml_dtypes.bfloat16
F32 = np.float32

SIZE = 11
KW = 2 * SIZE + 1  # 23
C1 = 0.01 ** 2
C2 = 0.03 ** 2
B, C, H, W = 16, 3, 512, 512
NCORES = 8
BPC = B // NCORES           # batches per core
PLANES = BPC * C            # 6 planes of [512, 512] per core
NBLK = H // 128             # 4 partition blocks per plane

AF = mybir.ActivationFunctionType
OP = mybir.AluOpType

BANDW = 2 * SIZE + 128  # 150: output span of one 128-row input chunk


def _build_bands(taps_bf: np.ndarray) -> np.ndarray:
    """Band matrix F [128, 150] bf16: F[i, j] = taps[i - j + 2*SIZE] (else 0).

    Chunk k of a 512-row map contributes lhsT_chunk.T @ F to output columns
    [128k-11, 128k+139) of the convolved map (Toeplitz shift invariance).
    """
    t = np.asarray(taps_bf, dtype=np.float64)
    F = np.zeros((128, BANDW))
    for i in range(128):
        for j in range(max(0, i), min(BANDW, i + KW)):
            F[i, j] = t[i - j + 2 * SIZE]
    return np.ascontiguousarray(F).astype(BF16)


def _emit_blur_pass(nc, src_sb, bands, out_psum_col0, psum_tile, blk):
    """One 1D-conv pass for one output partition-block `blk`.

    src_sb: [128, 2048] bf16 map, free dim chunk-major (4 x 512). Emits 7
    matmuls into psum_tile[:, out_psum_col0 : out_psum_col0+512] (one PSUM
    bank): chunk k covers output cols [128k-11, 128k+139) clipped to the
    bank; overlaps with the previous chunk accumulate. One accumulation
    group per bank: start on the first matmul, stop on the last.
    """
    col = lambda c: out_psum_col0 + c
    for k in range(4):
        lhsT = src_sb[:, 512 * k + 128 * blk: 512 * k + 128 * blk + 128]
        if k == 0:
            # cols [0, 139) <- F[:, 11:150]; opens the bank's group
            nc.tensor.matmul(psum_tile[:, col(0): col(139)],
                             lhsT, bands[:, 11:150], start=True, stop=False)
        else:
            # overlap cols [128k-11, 128k+11) <- F[:, 0:22] (accumulate)
            nc.tensor.matmul(
                psum_tile[:, col(128 * k - 11): col(128 * k + 11)],
                lhsT, bands[:, 0:22], start=False, stop=False)
            # fresh cols [128k+11, min(128k+139, 512)) <- F[:, 22:...]
            hi = min(128 * k + 139, 512)
            nc.tensor.matmul(
                psum_tile[:, col(128 * k + 11): col(hi)],
                lhsT, bands[:, 22: 22 + hi - (128 * k + 11)],
                start=False, stop=(k == 3))


def build_module(alpha: float, planes: int = PLANES):
    """Build the single-core Bass module (same program runs on all 8 cores)."""
    nc = bacc.Bacc("TRN2", target_bir_lowering=False, debug=False)
    bf = mybir.dt.bfloat16
    f32 = mybir.dt.float32

    x_dram = nc.dram_tensor("x", [planes, H, W], bf, kind="ExternalInput")
    y_dram = nc.dram_tensor("y", [planes, H, W], bf, kind="ExternalInput")
    bands_dram = nc.dram_tensor("bands", [128, BANDW], bf,
                                kind="ExternalInput")
    acc_dram = nc.dram_tensor("acc", [128, planes * NBLK], f32,
                              kind="ExternalOutput")

    with tile.TileContext(nc) as tc:
        with (
            tc.tile_pool(name="const", bufs=1) as const_pool,
            tc.tile_pool(name="io", bufs=2) as io_pool,
            tc.tile_pool(name="maps", bufs=2) as maps_pool,
            tc.tile_pool(name="umaps", bufs=2) as umaps_pool,
            tc.tile_pool(name="ep", bufs=2) as ep_pool,
            tc.tile_pool(name="pa", bufs=3, space="PSUM") as pa_pool,
            tc.tile_pool(name="pb", bufs=1, space="PSUM") as pb_pool,
        ):
            bands = const_pool.tile([128, BANDW], bf)
            nc.sync.dma_start(bands[:], bands_dram.ap())
            acc_sb = const_pool.tile([128, planes * NBLK], f32, tag="acc")

            for p in range(planes):
                # ---- load x, y plane as [128, 2048] (chunk-major free) ----
                x_t = io_pool.tile([128, 2048], bf, tag="x_t")
                y_t = io_pool.tile([128, 2048], bf, tag="y_t")
                for k in range(4):
                    nc.sync.dma_start(
                        x_t[:, 512 * k: 512 * k + 512],
                        x_dram.ap()[p, 128 * k: 128 * k + 128, :])
                    nc.sync.dma_start(
                        y_t[:, 512 * k: 512 * k + 512],
                        y_dram.ap()[p, 128 * k: 128 * k + 128, :])

                # ---- pre: the four conv inputs, bf16 ----
                u_t = maps_pool.tile([128, 2048], bf, tag="u")
                v_t = maps_pool.tile([128, 2048], bf, tag="v")
                d2_t = maps_pool.tile([128, 2048], bf, tag="d2")
                s2_t = maps_pool.tile([128, 2048], bf, tag="s2")
                x2_t = maps_pool.tile([128, 2048], bf, tag="x2")
                y2_t = maps_pool.tile([128, 2048], bf, tag="y2")
                nc.vector.tensor_tensor(u_t[:], x_t[:], y_t[:], OP.add)
                nc.vector.tensor_tensor(v_t[:], x_t[:], y_t[:], OP.subtract)
                nc.vector.scalar_tensor_tensor(
                    d2_t[:], x_t[:], 2.0, y_t[:], OP.mult, OP.mult)
                nc.gpsimd.tensor_tensor(x2_t[:], x_t[:], x_t[:], OP.mult)
                nc.gpsimd.tensor_tensor(y2_t[:], y_t[:], y_t[:], OP.mult)
                nc.gpsimd.tensor_tensor(s2_t[:], x2_t[:], y2_t[:], OP.add)

                # ---- pass A (H-direction blur, output transposed) ----
                srcs = [u_t, v_t, d2_t, s2_t]
                umaps = [umaps_pool.tile([128, 2048], bf, tag=f"U{i}",
                                         name=f"U{i}")
                         for i in range(4)]
                for mi, (src, um) in enumerate(zip(srcs, umaps)):
                    for blk in range(4):
                        ps_a = pa_pool.tile([128, 512], f32, tag="pa")
                        _emit_blur_pass(nc, src, bands, 0, ps_a, blk)
                        dst = um[:, 512 * blk: 512 * blk + 512]
                        if mi % 2 == 0:
                            nc.vector.tensor_copy(dst, ps_a[:])
                        else:
                            nc.scalar.copy(dst, ps_a[:])

                # ---- pass B (W-direction blur) + epilogue per block ----
                for blk in range(4):
                    ps_b = pb_pool.tile([128, 2048], f32, tag="pb")
                    for mi, um in enumerate(umaps):
                        _emit_blur_pass(nc, um, bands, 512 * mi, ps_b, blk)

                    # epilogue: ps_b = [ms | md | d2b | s2b]
                    # A,B kept fp32: bf16 here + bf16 g/h biases mean(ssim)
                    # by ~-1e-3 (see check_math bisection); fp32 is free on ACT
                    s_ab = ep_pool.tile([128, 1024], f32, tag="s_ab")
                    nc.scalar.activation(
                        s_ab[:], ps_b[:, 0:1024], AF.Square, scale=alpha)
                    s_t = ep_pool.tile([128, 1024], bf, tag="s_t")
                    nc.vector.tensor_scalar(
                        s_t[:], ps_b[:, 1024:2048],
                        2.0 * alpha, 2.0 * C2, OP.mult, OP.add)
                    gh = ep_pool.tile([128, 1024], bf, tag="gh")
                    nc.gpsimd.tensor_tensor(
                        gh[:, 0:512], s_ab[:, 0:512], s_ab[:, 512:1024],
                        OP.subtract)
                    nc.gpsimd.tensor_tensor(
                        gh[:, 512:1024], s_ab[:, 0:512], s_ab[:, 512:1024],
                        OP.add)
                    nd2 = ep_pool.tile([128, 1024], bf, tag="nd2")
                    nc.vector.tensor_tensor(nd2[:], s_t[:], gh[:], OP.subtract)
                    numden = ep_pool.tile([128, 1024], bf, tag="numden")
                    nc.vector.scalar_tensor_tensor(
                        numden[:], gh[:], 2.0 * C1, nd2[:], OP.add, OP.mult)
                    lnden = ep_pool.tile([128, 512], f32, tag="lnden")
                    nc.scalar.activation(
                        lnden[:], numden[:, 512:1024], AF.Ln)
                    rden = ep_pool.tile([128, 512], bf, tag="rden")
                    nc.scalar.activation(rden[:], lnden[:], AF.Exp, scale=-1.0)
                    # tensor_tensor_reduce dies on HW via the axon/PJRT path;
                    # scalar_tensor_tensor's accum_out is the standard-ISA way
                    ssim_t = ep_pool.tile([128, 512], bf, tag="ssim")
                    idx = p * NBLK + blk
                    nc.vector.scalar_tensor_tensor(
                        ssim_t[:], numden[:, 0:512], 1.0, rden[:],
                        OP.mult, OP.mult,
                        accum_out=acc_sb[:, idx:idx + 1])

            nc.sync.dma_start(acc_dram.ap(), acc_sb[:])
    return nc


_CACHE = {}


def _get_module(alpha: float):
    key = round(alpha, 12)
    if key not in _CACHE:
        nc = build_module(alpha)
        nc.compile()
        _CACHE[key] = nc
    return _CACHE[key]


def kernel(input, target, weight, _trace=False):
    input = np.asarray(input)
    target = np.asarray(target)
    weight = np.asarray(weight)

    # exact rank-1 factor of the Gaussian: rows of weight sum to g_i/sum(g)
    taps = np.asarray(weight[0, 0], dtype=np.float64).sum(axis=1)
    taps_bf = taps.astype(BF16)
    s = float(np.asarray(taps_bf, np.float64).sum())
    alpha = float(1.0 / (s * s))
    bands = _build_bands(taps_bf)

    nc = _get_module(alpha)

    in_maps = []
    for c in range(NCORES):
        xs = input[c * BPC:(c + 1) * BPC].reshape(PLANES, H, W).astype(BF16)
        ys = target[c * BPC:(c + 1) * BPC].reshape(PLANES, H, W).astype(BF16)
        in_maps.append({
            "x": np.ascontiguousarray(xs),
            "y": np.ascontiguousarray(ys),
            "bands": bands,
        })

    res = run_bass_kernel_spmd(
        nc, in_maps, core_ids=list(range(NCORES)), trace=_trace)

    total = 0.0
    for c in range(NCORES):
        total += np.asarray(res.results[c]["acc"], dtype=np.float64).sum()
    loss = 1.0 - total / float(B * C * H * W)
    out = np.float32(loss)
    if _trace:
        return out, res
    return out


# revision 12
# speedup vs baseline: 1.0032x; 1.0032x over previous
"""SSIM loss Bass/Tile kernel for Trainium2, data-parallel over 8 NeuronCores.

Math: the reference's 23x23 Gaussian depthwise conv is exactly separable
(weight = outer(g, g) with g = weight.sum(axis=1)), so each 2D blur is two
1D 23-tap convolutions, each expressed as banded-Toeplitz matmuls on the
tensor engine ("data as lhsT": out = data_chunk.T @ band, which convolves
the partition dim and transposes the layout; two passes restore layout).

Blurred maps per plane (4): ms=blur(x+y), md=blur(x-y), d2b=blur(2xy),
s2b=blur(x^2+y^2). Then with A=ms^2, B=md^2 (scaled to undo bf16 tap-sum
error): N1=A-B+2C1, D1=A+B+2C1, N2=2*d2b+2C2-(A-B), D2=2*s2b+2C2-(A+B)
give ssim = N1*N2/(D1*D2) (all terms are 2x the reference's, which cancels).
Per-core partial sums are reduced on host: loss = 1 - sum/count.
"""

import numpy as np
import ml_dtypes

import concourse.bass as bass
import concourse.tile as tile
from concourse import bacc, mybir
from concourse.bass_utils import run_bass_kernel_spmd

BF16 = ml_dtypes.bfloat16
F32 = np.float32

SIZE = 11
KW = 2 * SIZE + 1  # 23
C1 = 0.01 ** 2
C2 = 0.03 ** 2
B, C, H, W = 16, 3, 512, 512
NCORES = 8
BPC = B // NCORES           # batches per core
PLANES = BPC * C            # 6 planes of [512, 512] per core
NBLK = H // 128             # 4 partition blocks per plane

AF = mybir.ActivationFunctionType
OP = mybir.AluOpType

BANDW = 2 * SIZE + 128  # 150: output span of one 128-row input chunk


def _build_bands(taps_bf: np.ndarray) -> np.ndarray:
    """Band matrix F [128, 150] bf16: F[i, j] = taps[i - j + 2*SIZE] (else 0).

    Chunk k of a 512-row map contributes lhsT_chunk.T @ F to output columns
    [128k-11, 128k+139) of the convolved map (Toeplitz shift invariance).
    """
    t = np.asarray(taps_bf, dtype=np.float64)
    F = np.zeros((128, BANDW))
    for i in range(128):
        for j in range(max(0, i), min(BANDW, i + KW)):
            F[i, j] = t[i - j + 2 * SIZE]
    return np.ascontiguousarray(F).astype(BF16)


def _emit_blur_pass(nc, src_sb, bands, out_psum_col0, psum_tile, blk):
    """One 1D-conv pass for one output partition-block `blk`.

    src_sb: [128, 2048] bf16 map, free dim chunk-major (4 x 512). Emits 7
    matmuls into psum_tile[:, out_psum_col0 : out_psum_col0+512] (one PSUM
    bank): chunk k covers output cols [128k-11, 128k+139) clipped to the
    bank; overlaps with the previous chunk accumulate. One accumulation
    group per bank: start on the first matmul, stop on the last.
    """
    col = lambda c: out_psum_col0 + c
    for k in range(4):
        lhsT = src_sb[:, 512 * k + 128 * blk: 512 * k + 128 * blk + 128]
        if k == 0:
            # cols [0, 139) <- F[:, 11:150]; opens the bank's group
            nc.tensor.matmul(psum_tile[:, col(0): col(139)],
                             lhsT, bands[:, 11:150], start=True, stop=False)
        else:
            # overlap cols [128k-11, 128k+11) <- F[:, 0:22] (accumulate)
            nc.tensor.matmul(
                psum_tile[:, col(128 * k - 11): col(128 * k + 11)],
                lhsT, bands[:, 0:22], start=False, stop=False)
            # fresh cols [128k+11, min(128k+139, 512)) <- F[:, 22:...]
            hi = min(128 * k + 139, 512)
            nc.tensor.matmul(
                psum_tile[:, col(128 * k + 11): col(hi)],
                lhsT, bands[:, 22: 22 + hi - (128 * k + 11)],
                start=False, stop=(k == 3))


def build_module(alpha: float, planes: int = PLANES, repeat: int = 1):
    """Build the single-core Bass module (same program runs on all 8 cores).

    repeat > 1 re-emits the whole plane loop (same acc slots) — used only
    to measure per-iteration device time free of host/RPC overhead.
    """
    nc = bacc.Bacc("TRN2", target_bir_lowering=False, debug=False)
    bf = mybir.dt.bfloat16
    f32 = mybir.dt.float32

    x_dram = nc.dram_tensor("x", [planes, H, W], bf, kind="ExternalInput")
    y_dram = nc.dram_tensor("y", [planes, H, W], bf, kind="ExternalInput")
    bands_dram = nc.dram_tensor("bands", [128, BANDW], bf,
                                kind="ExternalInput")
    acc_dram = nc.dram_tensor("acc", [128, planes * NBLK], f32,
                              kind="ExternalOutput")

    with tile.TileContext(nc) as tc:
        with (
            tc.tile_pool(name="const", bufs=1) as const_pool,
            tc.tile_pool(name="io", bufs=2) as io_pool,
            tc.tile_pool(name="maps", bufs=2) as maps_pool,
            tc.tile_pool(name="umaps", bufs=2) as umaps_pool,
            tc.tile_pool(name="ep", bufs=2) as ep_pool,
            tc.tile_pool(name="pa", bufs=3, space="PSUM") as pa_pool,
            tc.tile_pool(name="pb", bufs=1, space="PSUM") as pb_pool,
        ):
            bands = const_pool.tile([128, BANDW], bf)
            nc.sync.dma_start(bands[:], bands_dram.ap())
            acc_sb = const_pool.tile([128, planes * NBLK], f32, tag="acc")

            for p in [pp for _ in range(repeat) for pp in range(planes)]:
                # ---- load x, y plane as [128, 2048] (chunk-major free) ----
                x_t = io_pool.tile([128, 2048], bf, tag="x_t")
                y_t = io_pool.tile([128, 2048], bf, tag="y_t")
                for k in range(4):
                    nc.sync.dma_start(
                        x_t[:, 512 * k: 512 * k + 512],
                        x_dram.ap()[p, 128 * k: 128 * k + 128, :])
                    nc.sync.dma_start(
                        y_t[:, 512 * k: 512 * k + 512],
                        y_dram.ap()[p, 128 * k: 128 * k + 128, :])

                # ---- pre: the four conv inputs, bf16 ----
                u_t = maps_pool.tile([128, 2048], bf, tag="u")
                v_t = maps_pool.tile([128, 2048], bf, tag="v")
                d2_t = maps_pool.tile([128, 2048], bf, tag="d2")
                s2_t = maps_pool.tile([128, 2048], bf, tag="s2")
                x2_t = maps_pool.tile([128, 2048], bf, tag="x2")
                y2_t = maps_pool.tile([128, 2048], bf, tag="y2")
                nc.vector.tensor_tensor(u_t[:], x_t[:], y_t[:], OP.add)
                nc.vector.tensor_tensor(v_t[:], x_t[:], y_t[:], OP.subtract)
                nc.vector.scalar_tensor_tensor(
                    d2_t[:], x_t[:], 2.0, y_t[:], OP.mult, OP.mult)
                nc.gpsimd.tensor_tensor(x2_t[:], x_t[:], x_t[:], OP.mult)
                nc.gpsimd.tensor_tensor(y2_t[:], y_t[:], y_t[:], OP.mult)
                nc.gpsimd.tensor_tensor(s2_t[:], x2_t[:], y2_t[:], OP.add)

                # ---- pass A (H-direction blur, output transposed) ----
                srcs = [u_t, v_t, d2_t, s2_t]
                umaps = [umaps_pool.tile([128, 2048], bf, tag=f"U{i}",
                                         name=f"U{i}")
                         for i in range(4)]
                for mi, (src, um) in enumerate(zip(srcs, umaps)):
                    for blk in range(4):
                        ps_a = pa_pool.tile([128, 512], f32, tag="pa")
                        _emit_blur_pass(nc, src, bands, 0, ps_a, blk)
                        dst = um[:, 512 * blk: 512 * blk + 512]
                        if mi % 2 == 0:
                            nc.vector.tensor_copy(dst, ps_a[:])
                        else:
                            nc.scalar.copy(dst, ps_a[:])

                # ---- pass B (W-direction blur) + epilogue per block ----
                for blk in range(4):
                    ps_b = pb_pool.tile([128, 2048], f32, tag="pb")
                    for mi, um in enumerate(umaps):
                        _emit_blur_pass(nc, um, bands, 512 * mi, ps_b, blk)

                    # epilogue: ps_b = [ms | md | d2b | s2b]
                    # A,B kept fp32: bf16 here + bf16 g/h biases mean(ssim)
                    # by ~-1e-3 (see check_math bisection); fp32 is free on ACT
                    s_ab = ep_pool.tile([128, 1024], f32, tag="s_ab")
                    nc.scalar.activation(
                        s_ab[:], ps_b[:, 0:1024], AF.Square, scale=alpha)
                    s_t = ep_pool.tile([128, 1024], bf, tag="s_t")
                    nc.vector.tensor_scalar(
                        s_t[:], ps_b[:, 1024:2048],
                        2.0 * alpha, 2.0 * C2, OP.mult, OP.add)
                    gh = ep_pool.tile([128, 1024], bf, tag="gh")
                    nc.gpsimd.tensor_tensor(
                        gh[:, 0:512], s_ab[:, 0:512], s_ab[:, 512:1024],
                        OP.subtract)
                    nc.gpsimd.tensor_tensor(
                        gh[:, 512:1024], s_ab[:, 0:512], s_ab[:, 512:1024],
                        OP.add)
                    nd2 = ep_pool.tile([128, 1024], bf, tag="nd2")
                    nc.vector.tensor_tensor(nd2[:], s_t[:], gh[:], OP.subtract)
                    numden = ep_pool.tile([128, 1024], bf, tag="numden")
                    nc.vector.scalar_tensor_tensor(
                        numden[:], gh[:], 2.0 * C1, nd2[:], OP.add, OP.mult)
                    lnden = ep_pool.tile([128, 512], f32, tag="lnden")
                    nc.scalar.activation(
                        lnden[:], numden[:, 512:1024], AF.Ln)
                    rden = ep_pool.tile([128, 512], bf, tag="rden")
                    nc.scalar.activation(rden[:], lnden[:], AF.Exp, scale=-1.0)
                    # tensor_tensor_reduce dies on HW via the axon/PJRT path;
                    # scalar_tensor_tensor's accum_out is the standard-ISA way
                    ssim_t = ep_pool.tile([128, 512], bf, tag="ssim")
                    idx = p * NBLK + blk
                    nc.vector.scalar_tensor_tensor(
                        ssim_t[:], numden[:, 0:512], 1.0, rden[:],
                        OP.mult, OP.mult,
                        accum_out=acc_sb[:, idx:idx + 1])

            nc.sync.dma_start(acc_dram.ap(), acc_sb[:])
    return nc


_CACHE = {}


def _get_module(alpha: float):
    key = round(alpha, 12)
    if key not in _CACHE:
        nc = build_module(alpha)
        nc.compile()
        _CACHE[key] = nc
    return _CACHE[key]


def kernel(input, target, weight, _trace=False):
    input = np.asarray(input)
    target = np.asarray(target)
    weight = np.asarray(weight)

    # exact rank-1 factor of the Gaussian: rows of weight sum to g_i/sum(g)
    taps = np.asarray(weight[0, 0], dtype=np.float64).sum(axis=1)
    taps_bf = taps.astype(BF16)
    s = float(np.asarray(taps_bf, np.float64).sum())
    alpha = float(1.0 / (s * s))
    bands = _build_bands(taps_bf)

    nc = _get_module(alpha)

    in_maps = []
    for c in range(NCORES):
        xs = input[c * BPC:(c + 1) * BPC].reshape(PLANES, H, W).astype(BF16)
        ys = target[c * BPC:(c + 1) * BPC].reshape(PLANES, H, W).astype(BF16)
        in_maps.append({
            "x": np.ascontiguousarray(xs),
            "y": np.ascontiguousarray(ys),
            "bands": bands,
        })

    res = run_bass_kernel_spmd(
        nc, in_maps, core_ids=list(range(NCORES)), trace=_trace)

    total = 0.0
    for c in range(NCORES):
        total += np.asarray(res.results[c]["acc"], dtype=np.float64).sum()
    loss = 1.0 - total / float(B * C * H * W)
    out = np.float32(loss)
    if _trace:
        return out, res
    return out
